# revision 19
# baseline (speedup 1.0000x reference)
"""Causal self-attention (B=2, T=2048, C=768, H=12) on 8 NeuronCores.

Sharding per the hint: data-parallel on B (cores 0-3 batch 0, 4-7 batch 1)
x tensor-parallel over heads (core d%4 owns heads 3(d%4)..3(d%4)+2). Each
core computes q/k/v for ONLY its 3 heads, its heads' full causal TxT
attention, and a PARTIAL output projection (contraction over its 192 Y
columns). The 4 partials per batch are summed on the host (pure gather +
add) - zero device collectives.

Per-core pipeline (bf16 matmul operands, fp32 PSUM accumulation):
  x^T arrives pre-transposed from host (no on-device transposes at all).
  K^T/Q^T generated in 3 stationary-weight pair-matmuls ([128,2048] each:
  (k0|k1), (k2|q0), (q1|q2)), V in t-major [128, 3, 65] with a folded
  ones-row for the softmax denominator. Attention runs qtile-major
  (256-query tiles) with EXACT causal extents: qtile i processes step
  pairs sp=0..i, each [128, 512] = two 128-key tiles; only the last
  (diagonal) pair needs a mask multiply. exp on ACT (scale=1/8, no
  max-sub), P^T @ [V|1] accumulates [65, 256] per head into a shared
  [65, 768] PSUM tile, reciprocal + ones-matmul broadcast normalizes into
  Y^T, then the projection partial for the qtile's two 128-row t-tiles is
  copied to bf16 and DMAd out. K/Q/V generation is interleaved with the
  qtile loop so ACT/DVE spin up ~6us into the kernel.
"""

import numpy as np
import ml_dtypes

B, T, C, H, D = 2, 2048, 768, 12, 64
NCORES = 8
HPC = 3            # heads per core
QTW = 256          # query tile width
NQT = T // QTW     # 8 query tiles
CT = C // 128      # 6 contraction tiles

_CACHE = {}

# head -> (pair tile index, partition offset) for K^T and Q^T slices.
# pair tiles: 0 = (k0|k1), 1 = (q0|q1), 2 = (k2|q2); tile 3 = q2 copy [64,T]
_KSL = [(0, 0), (0, 64), (2, 0)]
_QSL = [(1, 0), (1, 64), (3, 0)]


def _build_program(with_bias=True):
    import concourse.bass as bass
    import concourse.bacc as bacc
    import concourse.mybir as mybir
    import concourse.tile as tile

    F32 = mybir.dt.float32
    BF16 = mybir.dt.bfloat16
    AF = mybir.ActivationFunctionType

    nc = bacc.Bacc()
    xT_in = nc.declare_dram_parameter("xT", [CT, 128, T], BF16, isOutput=False)
    wkq_in = nc.declare_dram_parameter("wkq", [CT, 128, 384], BF16,
                                       isOutput=False)
    wv_in = nc.declare_dram_parameter("wv", [CT, 128, 192], BF16,
                                      isOutput=False)
    wp_in = nc.declare_dram_parameter("wp", [2, 128, C], BF16, isOutput=False)
    masks_in = nc.declare_dram_parameter("masks", [128, 512], BF16,
                                         isOutput=False)
    if with_bias:
        bkq_in = nc.declare_dram_parameter("bkq", [128, 3], F32,
                                           isOutput=False)
    z_out = nc.declare_dram_parameter("z", [T, C], BF16, isOutput=True)

    scale = 1.0 / float(np.sqrt(D))

    with tile.TileContext(nc) as tc:
        with tc.tile_pool(name="const", bufs=1) as constp, \
             tc.tile_pool(name="data", bufs=1) as datap, \
             tc.tile_pool(name="pt", bufs=3) as ptp, \
             tc.tile_pool(name="small", bufs=3) as smallp, \
             tc.tile_pool(name="zs", bufs=2) as zsp, \
             tc.tile_pool(name="ps", bufs=4, space="PSUM") as psp, \
             tc.tile_pool(name="pot", bufs=2, space="PSUM") as potp:

            # ---- constants ------------------------------------------------
            wkq_s = constp.tile([128, CT, 384], BF16, tag="wkq")
            wv_s = constp.tile([128, CT, 192], BF16, tag="wv")
            wp_s = constp.tile([128, 2, C], BF16, tag="wp")
            masks_s = constp.tile([128, 512], BF16, tag="masks")
            ones1 = constp.tile([1, 64], BF16, tag="ones1")
            nc.vector.memset(ones1, 1.0)
            if with_bias:
                bkq_s = constp.tile([128, 3], F32, tag="bkq")

            # ---- persistent data ------------------------------------------
            xT = [datap.tile([128, T], BF16, tag=f"xT{c}", name=f"xT{c}")
                  for c in range(CT)]
            KQ = [datap.tile([128, T], BF16, tag=f"KQ{j}", name=f"KQ{j}")
                  for j in range(3)]
            KQ.append(datap.tile([64, T], BF16, tag="KQ3", name="KQ3"))
            V = [datap.tile([128, HPC, D + 1], BF16, tag=f"V{t}",
                            name=f"V{t}") for t in range(T // 128)]
            YT0 = datap.tile([128, T], BF16, tag="YT0", name="YT0")
            YT1 = datap.tile([64, T], BF16, tag="YT1", name="YT1")

            # ---- input DMAs (priority order) ------------------------------
            # scalar queue: weights + masks; sync queue: xT halves
            nc.scalar.dma_start(
                out=wkq_s,
                in_=bass.AP(tensor=wkq_in[:, :, :].tensor,
                            offset=wkq_in[:, :, :].offset,
                            ap=[[384, 128], [128 * 384, CT], [1, 384]]))
            for half in range(2):
                lo = (T // 2) * half
                for c in range(CT):
                    nc.sync.dma_start(
                        out=xT[c][:, lo:lo + T // 2],
                        in_=xT_in[c, :, lo:lo + T // 2])
            nc.scalar.dma_start(
                out=wv_s,
                in_=bass.AP(tensor=wv_in[:, :, :].tensor,
                            offset=wv_in[:, :, :].offset,
                            ap=[[192, 128], [128 * 192, CT], [1, 192]]))
            nc.scalar.dma_start(out=masks_s, in_=masks_in[:, :])
            nc.scalar.dma_start(
                out=wp_s,
                in_=bass.AP(tensor=wp_in[:, :, :].tensor,
                            offset=wp_in[:, :, :].offset,
                            ap=[[C, 128], [128 * C, 2], [1, C]]))
            if with_bias:
                nc.gpsimd.dma_start(out=bkq_s, in_=bkq_in[:, :])

            def gen_kq(arg):
                tcnk, j = arg
                lo = 512 * tcnk
                acc = psp.tile([128, 512], F32, tag="acc", name="acc")
                for c in range(CT):
                    nc.tensor.matmul(
                        out=acc,
                        lhsT=wkq_s[:, c, 128 * j:128 * (j + 1)],
                        rhs=xT[c][:, lo:lo + 512],
                        start=(c == 0), stop=(c == CT - 1))
                if with_bias:
                    nc.vector.tensor_scalar_add(
                        KQ[j][:, lo:lo + 512], in0=acc,
                        scalar1=bkq_s[:, j:j + 1])
                else:
                    nc.vector.tensor_copy(out=KQ[j][:, lo:lo + 512],
                                          in_=acc)
                if j == 2:
                    # peel q2 (partitions 64:128 of the (k2|q2) pair) into a
                    # base-0 tile so S(h2) operands share a base partition
                    nc.vector.tensor_copy(out=KQ[3][0:64, lo:lo + 512],
                                          in_=KQ[2][64:128, lo:lo + 512])

            def gen_v(t):
                acc = psp.tile([128, 512], F32, tag="acc", name="accv")
                for c in range(CT):
                    nc.tensor.matmul(
                        out=acc[:, 0:192],
                        lhsT=xT[c][:, 128 * t:128 * (t + 1)],
                        rhs=wv_s[:, c, :],
                        start=(c == 0), stop=(c == CT - 1))
                nc.vector.tensor_copy(out=V[t][:, :, 0:D], in_=acc[:, 0:192])
                nc.vector.memset(V[t][:, :, D:D + 1], 1.0)

            # ---- main loop: generation interleaved into attention rounds --
            # gen(tc) must land before qtile 2*tc; tc0 runs up front, tc>=1
            # is spread one-group-per-sp-round across qtiles 2tc-2, 2tc-1.
            for j in range(3):
                gen_kq((0, j))
            for t in range(4):
                gen_v(t)
            gen_sched = {}
            for tcn in range(1, 4):
                thunks = ([(gen_kq, (tcn, j)) for j in range(3)] +
                          [(gen_v, t) for t in range(4 * tcn, 4 * tcn + 4)])
                qa, qb = 2 * tcn - 2, 2 * tcn - 1
                na = min(len(thunks), max(1, qa + 1))
                gen_sched[qa] = thunks[:na]
                gen_sched[qb] = thunks[na:]

            for i in range(NQT):
                pending = list(gen_sched.get(i, []))
                per_round = max(1, -(-len(pending) // (i + 1)))
                qsl = slice(QTW * i, QTW * (i + 1))
                ot = potp.tile([65, 3 * QTW], F32, tag="ot", name="ot")
                pts = {}
                for sp in range(i + 1):
                    for h in range(HPC):
                        jk, pk = _KSL[h]
                        jq, pq = _QSL[h]
                        sps = psp.tile([128, 512], F32, tag="acc",
                                       name="sps")
                        for half in range(2):
                            klo = 256 * sp + 128 * half
                            nc.tensor.matmul(
                                out=sps[:, 256 * half:256 * (half + 1)],
                                lhsT=KQ[jk][pk:pk + 64, klo:klo + 128],
                                rhs=KQ[jq][pq:pq + 64, qsl],
                                start=True, stop=True)
                        pt = ptp.tile([128, 512], BF16, tag="pt", name="pt")
                        nc.scalar.activation(out=pt, in_=sps, func=AF.Exp,
                                             scale=scale)
                        if sp == i:
                            nc.vector.tensor_mul(pt, pt, masks_s)
                        pts[h] = pt
                    for h in range(HPC):
                        hsl = slice(QTW * h, QTW * (h + 1))
                        for half in range(2):
                            nc.tensor.matmul(
                                out=ot[:, hsl],
                                lhsT=V[2 * sp + half][:, h, :],
                                rhs=pts[h][:, 256 * half:256 * (half + 1)],
                                start=(sp == 0 and half == 0),
                                stop=(sp == i and half == 1),
                                skip_group_check=True)
                    for _ in range(per_round):
                        if pending:
                            fn, arg = pending.pop(0)
                            fn(arg)
                for h in range(HPC):
                    hsl = slice(QTW * h, QTW * (h + 1))
                    rec = smallp.tile([1, QTW], F32, tag="rec", name="rec")
                    nc.vector.reciprocal(out=rec, in_=ot[64:65, hsl])
                    recbf = smallp.tile([1, QTW], BF16, tag="recbf",
                                        name="recbf")
                    nc.vector.tensor_copy(out=recbf, in_=rec)
                    recb = psp.tile([64, QTW], F32, tag="acc", name="recb")
                    nc.tensor.matmul(out=recb, lhsT=ones1, rhs=recbf,
                                     start=True, stop=True)
                    recb_sb = smallp.tile([64, QTW], F32, tag="recb_sb",
                                          name="recb_sb")
                    nc.vector.tensor_copy(out=recb_sb, in_=recb)
                    ysl = (YT0[0:64, qsl] if h == 0 else
                           YT0[64:128, qsl] if h == 1 else YT1[0:64, qsl])
                    nc.vector.tensor_mul(ysl, ot[0:64, hsl], recb_sb)

                for tt in (2 * i, 2 * i + 1):
                    tsl = slice(128 * tt, 128 * (tt + 1))
                    zt = zsp.tile([128, C], BF16, tag="zt", name="zt")
                    for ph in range(2):
                        csl = slice(384 * ph, 384 * (ph + 1))
                        pacc = psp.tile([128, 384], F32, tag="acc",
                                        name="pacc")
                        nc.tensor.matmul(out=pacc, lhsT=YT0[:, tsl],
                                         rhs=wp_s[:, 0, csl], start=True,
                                         stop=False)
                        nc.tensor.matmul(out=pacc, lhsT=YT1[:, tsl],
                                         rhs=wp_s[0:64, 1, csl],
                                         start=False, stop=True)
                        nc.vector.tensor_copy(out=zt[:, csl], in_=pacc)
                    nc.gpsimd.dma_start(out=z_out[tsl, :], in_=zt)

    nc.finalize()
    return nc


def _prep_inputs(x, W_qkv, b_qkv, W_proj, b_proj):
    bf16 = ml_dtypes.bfloat16
    x = np.asarray(x, dtype=np.float32)
    W_qkv = np.asarray(W_qkv, dtype=np.float32)
    b_qkv = np.asarray(b_qkv, dtype=np.float32)

    # masks [128, 512]: col 256*d + q valid iff q >= 128*d + p
    p = np.arange(128)[:, None]
    q = np.arange(QTW)[None, :]
    m = np.ones((128, 512), dtype=np.float32)
    m[:, 0:QTW] = q >= p
    m[:, QTW:512] = q >= 128 + p
    m_bf = np.ascontiguousarray(m.astype(bf16))

    xTb = [np.ascontiguousarray(
        x[b].T.astype(bf16).reshape(CT, 128, T)) for b in range(B)]

    in_maps = []
    for d in range(NCORES):
        b, g = d // 4, d % 4
        qcols = W_qkv[:, 192 * g:192 * (g + 1)]
        kcols = W_qkv[:, C + 192 * g:C + 192 * (g + 1)]
        vcols = W_qkv[:, 2 * C + 192 * g:2 * C + 192 * (g + 1)]
        wkq = np.concatenate(
            [kcols[:, 0:128], qcols[:, 0:128], kcols[:, 128:192],
             qcols[:, 128:192]], axis=1)         # [768, 384]
        wp = np.zeros((256, C), dtype=np.float32)
        wp[0:192] = W_proj[192 * g:192 * (g + 1), :]
        qb = b_qkv[192 * g:192 * (g + 1)]
        kb = b_qkv[C + 192 * g:C + 192 * (g + 1)]
        bkq = np.stack([kb[0:128], qb[0:128],
                        np.concatenate([kb[128:192], qb[128:192]])],
                       axis=1)                   # [128, 3]
        in_maps.append({
            "xT": xTb[b],
            "wkq": np.ascontiguousarray(wkq.astype(bf16).reshape(CT, 128, 384)),
            "wv": np.ascontiguousarray(vcols.astype(bf16).reshape(CT, 128, 192)),
            "wp": np.ascontiguousarray(wp.astype(bf16).reshape(2, 128, C)),
            "masks": m_bf,
            "bkq": np.ascontiguousarray(bkq.astype(np.float32)),
        })
    return in_maps


def kernel(x, W_qkv, b_qkv, W_proj, b_proj):
    import os
    from concourse.bass_utils import run_bass_kernel_spmd

    b_qkv = np.asarray(b_qkv, dtype=np.float32)
    b_proj = np.asarray(b_proj, dtype=np.float32)
    W_qkv = np.asarray(W_qkv, dtype=np.float32)
    W_proj = np.asarray(W_proj, dtype=np.float32)
    in_maps = _prep_inputs(x, W_qkv, b_qkv, W_proj, b_proj)
    with_bias = bool(np.any(b_qkv[0:2 * C]))
    if not with_bias:
        for im in in_maps:
            del im["bkq"]
    key = f"nc{with_bias}"
    if key not in _CACHE:
        _CACHE[key] = _build_program(with_bias)
    nc = _CACHE[key]
    res = run_bass_kernel_spmd(nc, in_maps, list(range(NCORES)),
                               trace=os.environ.get("KTRACE", "") == "1")
    _CACHE["last_result"] = res

    # host-side unshard: sum the 4 head-group partials per batch.
    out = np.empty((B, T, C), dtype=np.float32)
    for b in range(B):
        acc = np.zeros((T, C), dtype=np.float32)
        for g in range(4):
            acc += np.asarray(res.results[4 * b + g]["z"]).astype(np.float32)
        # v-bias and proj-bias fold in linearly on the host:
        # out = P(V + bv) Wp + bp = (PV) Wp + bv Wp + bp
        bv = b_qkv[2 * C:3 * C]
        out[b] = acc + (bv @ W_proj + b_proj)[None, :]
    return out


# revision 25
# speedup vs baseline: 1.2823x; 1.2823x over previous
"""Causal self-attention (B=2, T=2048, C=768, H=12) on 8 NeuronCores.

Sharding per the hint: data-parallel on B (cores 0-3 batch 0, 4-7 batch 1)
x tensor-parallel over heads (core d%4 owns heads 3(d%4)..3(d%4)+2). Each
core computes q/k/v for ONLY its 3 heads, its heads' full causal TxT
attention, and a PARTIAL output projection (contraction over its 192 Y
columns). The 4 partials per batch are summed on the host (pure gather +
add) - zero device collectives.

Per-core pipeline (bf16 matmul operands, fp32 PSUM accumulation):
  x^T arrives pre-transposed from host (no on-device transposes at all).
  K^T/Q^T generated in 3 stationary-weight pair-matmuls ([128,2048] each:
  (k0|k1), (k2|q0), (q1|q2)), V in t-major [128, 3, 65] with a folded
  ones-row for the softmax denominator. Attention runs qtile-major
  (256-query tiles) with EXACT causal extents: qtile i processes step
  pairs sp=0..i, each [128, 512] = two 128-key tiles; only the last
  (diagonal) pair needs a mask multiply. exp on ACT (scale=1/8, no
  max-sub), P^T @ [V|1] accumulates [65, 256] per head into a shared
  [65, 768] PSUM tile, reciprocal + ones-matmul broadcast normalizes into
  Y^T, then the projection partial for the qtile's two 128-row t-tiles is
  copied to bf16 and DMAd out. K/Q/V generation is interleaved with the
  qtile loop so ACT/DVE spin up ~6us into the kernel.
"""

import numpy as np
import ml_dtypes

B, T, C, H, D = 2, 2048, 768, 12, 64
NCORES = 8
HPC = 3            # heads per core
QTW = 256          # query tile width
NQT = T // QTW     # 8 query tiles
CT = C // 128      # 6 contraction tiles

_CACHE = {}

# head -> (pair tile index, partition offset) for K^T and Q^T slices.
# pair tiles: 0 = (k0|k1), 1 = (q0|q1), 2 = (k2|q2); tile 3 = q2 copy [64,T]
_KSL = [(0, 0), (0, 64), (2, 0)]
_QSL = [(1, 0), (1, 64), (3, 0)]


def _build_program(with_bias=True):
    import concourse.bass as bass
    import concourse.bacc as bacc
    import concourse.mybir as mybir
    import concourse.tile as tile

    F32 = mybir.dt.float32
    BF16 = mybir.dt.bfloat16
    AF = mybir.ActivationFunctionType

    nc = bacc.Bacc()
    xT_in = nc.declare_dram_parameter("xT", [CT, 128, T], BF16, isOutput=False)
    wkq_in = nc.declare_dram_parameter("wkq", [CT, 128, 384], BF16,
                                       isOutput=False)
    wv_in = nc.declare_dram_parameter("wv", [CT, 128, 192], BF16,
                                      isOutput=False)
    wp_in = nc.declare_dram_parameter("wp", [2, 128, C], BF16, isOutput=False)
    masks_in = nc.declare_dram_parameter("masks", [128, 512], BF16,
                                         isOutput=False)
    if with_bias:
        bkq_in = nc.declare_dram_parameter("bkq", [128, 3], F32,
                                           isOutput=False)
    z_out = nc.declare_dram_parameter("z", [T, C], BF16, isOutput=True)

    scale = 1.0 / float(np.sqrt(D))

    with tile.TileContext(nc) as tc:
        with tc.tile_pool(name="const", bufs=1) as constp, \
             tc.tile_pool(name="data", bufs=1) as datap, \
             tc.tile_pool(name="pt", bufs=3) as ptp, \
             tc.tile_pool(name="small", bufs=3) as smallp, \
             tc.tile_pool(name="zs", bufs=2) as zsp, \
             tc.tile_pool(name="ps", bufs=2, space="PSUM") as psp, \
             tc.tile_pool(name="pg", bufs=2, space="PSUM") as pgp, \
             tc.tile_pool(name="pot", bufs=2, space="PSUM") as potp:

            # ---- constants ------------------------------------------------
            wkq_s = constp.tile([128, CT, 384], BF16, tag="wkq")
            wv_s = constp.tile([128, CT, 192], BF16, tag="wv")
            wp_s = constp.tile([128, 2, C], BF16, tag="wp")
            masks_s = constp.tile([128, 512], BF16, tag="masks")
            ones1 = constp.tile([1, 64], BF16, tag="ones1")
            nc.vector.memset(ones1, 1.0)
            if with_bias:
                bkq_s = constp.tile([128, 3], F32, tag="bkq")

            # ---- persistent data ------------------------------------------
            xT = [datap.tile([128, T], BF16, tag=f"xT{c}", name=f"xT{c}")
                  for c in range(CT)]
            KQ = [datap.tile([128, T], BF16, tag=f"KQ{j}", name=f"KQ{j}")
                  for j in range(3)]
            KQ.append(datap.tile([64, T], BF16, tag="KQ3", name="KQ3"))
            V = [datap.tile([128, HPC, D + 1], BF16, tag=f"V{t}",
                            name=f"V{t}") for t in range(T // 128)]
            YT0 = datap.tile([128, T], BF16, tag="YT0", name="YT0")
            YT1 = datap.tile([64, T], BF16, tag="YT1", name="YT1")

            # ---- input DMAs (priority order) ------------------------------
            # scalar queue: weights + masks; sync queue: xT halves
            nc.scalar.dma_start(
                out=wkq_s,
                in_=bass.AP(tensor=wkq_in[:, :, :].tensor,
                            offset=wkq_in[:, :, :].offset,
                            ap=[[384, 128], [128 * 384, CT], [1, 384]]))
            for half in range(2):
                lo = (T // 2) * half
                for c in range(CT):
                    nc.sync.dma_start(
                        out=xT[c][:, lo:lo + T // 2],
                        in_=xT_in[c, :, lo:lo + T // 2])
            nc.scalar.dma_start(
                out=wv_s,
                in_=bass.AP(tensor=wv_in[:, :, :].tensor,
                            offset=wv_in[:, :, :].offset,
                            ap=[[192, 128], [128 * 192, CT], [1, 192]]))
            nc.scalar.dma_start(out=masks_s, in_=masks_in[:, :])
            nc.scalar.dma_start(
                out=wp_s,
                in_=bass.AP(tensor=wp_in[:, :, :].tensor,
                            offset=wp_in[:, :, :].offset,
                            ap=[[C, 128], [128 * C, 2], [1, C]]))
            if with_bias:
                nc.gpsimd.dma_start(out=bkq_s, in_=bkq_in[:, :])

            def gen_kq(arg):
                cn, j = arg
                lo = 256 * cn
                acc = pgp.tile([128, 256], F32, tag="acc", name="acc")
                for c in range(CT):
                    nc.tensor.matmul(
                        out=acc,
                        lhsT=wkq_s[:, c, 128 * j:128 * (j + 1)],
                        rhs=xT[c][:, lo:lo + 256],
                        start=(c == 0), stop=(c == CT - 1))
                if with_bias:
                    nc.vector.tensor_scalar_add(
                        KQ[j][:, lo:lo + 256], in0=acc,
                        scalar1=bkq_s[:, j:j + 1])
                else:
                    nc.vector.tensor_copy(out=KQ[j][:, lo:lo + 256],
                                          in_=acc)
                if j == 2:
                    # peel q2 (partitions 64:128 of the (k2|q2) pair) into a
                    # base-0 tile so S(h2) operands share a base partition
                    nc.vector.tensor_copy(out=KQ[3][0:64, lo:lo + 256],
                                          in_=KQ[2][64:128, lo:lo + 256])

            def gen_v(t):
                acc = pgp.tile([128, 256], F32, tag="acc", name="accv")
                for c in range(CT):
                    nc.tensor.matmul(
                        out=acc[:, 0:192],
                        lhsT=xT[c][:, 128 * t:128 * (t + 1)],
                        rhs=wv_s[:, c, :],
                        start=(c == 0), stop=(c == CT - 1))
                nc.vector.tensor_copy(out=V[t][:, :, 0:D], in_=acc[:, 0:192])
                nc.vector.memset(V[t][:, :, D:D + 1], 1.0)

            # ---- main loop: generation interleaved into attention rounds --
            # gen(tc) must land before qtile 2*tc; tc0 runs up front, tc>=1
            # is spread one-group-per-sp-round across qtiles 2tc-2, 2tc-1.
            for j in range(3):
                gen_kq((0, j))
            gen_v(0)
            gen_v(1)
            # qtile i consumes KQ chunk i and V[2i], V[2i+1]; emit chunk i+1
            # and its V pair during qtile i's rounds (one thunk per round).
            gen_sched = {
                i: ([(gen_kq, (i + 1, j)) for j in range(3)] +
                    [(gen_v, 2 * i + 2), (gen_v, 2 * i + 3)])
                for i in range(NQT - 1)
            }

            for i in range(NQT):
                pending = list(gen_sched.get(i, []))
                per_round = max(1, -(-len(pending) // (i + 1)))
                qsl = slice(QTW * i, QTW * (i + 1))
                ot = potp.tile([65, 3 * QTW], F32, tag="ot", name="ot")
                pts = {}
                for sp in range(i + 1):
                    for h in range(HPC):
                        jk, pk = _KSL[h]
                        jq, pq = _QSL[h]
                        sps = psp.tile([128, 512], F32, tag="acc",
                                       name="sps")
                        for half in range(2):
                            klo = 256 * sp + 128 * half
                            nc.tensor.matmul(
                                out=sps[:, 256 * half:256 * (half + 1)],
                                lhsT=KQ[jk][pk:pk + 64, klo:klo + 128],
                                rhs=KQ[jq][pq:pq + 64, qsl],
                                start=True, stop=True)
                        pt = ptp.tile([128, 512], BF16, tag="pt", name="pt")
                        nc.scalar.activation(out=pt, in_=sps, func=AF.Exp,
                                             scale=scale)
                        if sp == i:
                            nc.vector.tensor_mul(pt, pt, masks_s)
                        pts[h] = pt
                    for h in range(HPC):
                        hsl = slice(QTW * h, QTW * (h + 1))
                        for half in range(2):
                            nc.tensor.matmul(
                                out=ot[:, hsl],
                                lhsT=V[2 * sp + half][:, h, :],
                                rhs=pts[h][:, 256 * half:256 * (half + 1)],
                                start=(sp == 0 and half == 0),
                                stop=(sp == i and half == 1),
                                skip_group_check=True)
                    for _ in range(per_round):
                        if pending:
                            fn, arg = pending.pop(0)
                            fn(arg)
                for h in range(HPC):
                    hsl = slice(QTW * h, QTW * (h + 1))
                    rec = smallp.tile([1, QTW], F32, tag="rec", name="rec")
                    nc.vector.reciprocal(out=rec, in_=ot[64:65, hsl])
                    recbf = smallp.tile([1, QTW], BF16, tag="recbf",
                                        name="recbf")
                    nc.vector.tensor_copy(out=recbf, in_=rec)
                    recb = pgp.tile([64, QTW], F32, tag="acc", name="recb")
                    nc.tensor.matmul(out=recb, lhsT=ones1, rhs=recbf,
                                     start=True, stop=True)
                    recb_sb = smallp.tile([64, QTW], F32, tag="recb_sb",
                                          name="recb_sb")
                    nc.vector.tensor_copy(out=recb_sb, in_=recb)
                    ysl = (YT0[0:64, qsl] if h == 0 else
                           YT0[64:128, qsl] if h == 1 else YT1[0:64, qsl])
                    nc.vector.tensor_mul(ysl, ot[0:64, hsl], recb_sb)

                for tt in (2 * i, 2 * i + 1):
                    tsl = slice(128 * tt, 128 * (tt + 1))
                    zt = zsp.tile([128, C], BF16, tag="zt", name="zt")
                    for ph in range(2):
                        csl = slice(384 * ph, 384 * (ph + 1))
                        pacc = pgp.tile([128, 384], F32, tag="acc",
                                        name="pacc")
                        nc.tensor.matmul(out=pacc, lhsT=YT0[:, tsl],
                                         rhs=wp_s[:, 0, csl], start=True,
                                         stop=False)
                        nc.tensor.matmul(out=pacc, lhsT=YT1[:, tsl],
                                         rhs=wp_s[0:64, 1, csl],
                                         start=False, stop=True)
                        nc.vector.tensor_copy(out=zt[:, csl], in_=pacc)
                    nc.gpsimd.dma_start(out=z_out[tsl, :], in_=zt)

    nc.finalize()
    return nc


def _prep_inputs(x, W_qkv, b_qkv, W_proj, b_proj):
    bf16 = ml_dtypes.bfloat16
    x = np.asarray(x, dtype=np.float32)
    W_qkv = np.asarray(W_qkv, dtype=np.float32)
    b_qkv = np.asarray(b_qkv, dtype=np.float32)

    # masks [128, 512]: col 256*d + q valid iff q >= 128*d + p
    p = np.arange(128)[:, None]
    q = np.arange(QTW)[None, :]
    m = np.ones((128, 512), dtype=np.float32)
    m[:, 0:QTW] = q >= p
    m[:, QTW:512] = q >= 128 + p
    m_bf = np.ascontiguousarray(m.astype(bf16))

    xTb = [np.ascontiguousarray(
        x[b].T.astype(bf16).reshape(CT, 128, T)) for b in range(B)]

    in_maps = []
    for d in range(NCORES):
        b, g = d // 4, d % 4
        qcols = W_qkv[:, 192 * g:192 * (g + 1)]
        kcols = W_qkv[:, C + 192 * g:C + 192 * (g + 1)]
        vcols = W_qkv[:, 2 * C + 192 * g:2 * C + 192 * (g + 1)]
        wkq = np.concatenate(
            [kcols[:, 0:128], qcols[:, 0:128], kcols[:, 128:192],
             qcols[:, 128:192]], axis=1)         # [768, 384]
        wp = np.zeros((256, C), dtype=np.float32)
        wp[0:192] = W_proj[192 * g:192 * (g + 1), :]
        qb = b_qkv[192 * g:192 * (g + 1)]
        kb = b_qkv[C + 192 * g:C + 192 * (g + 1)]
        bkq = np.stack([kb[0:128], qb[0:128],
                        np.concatenate([kb[128:192], qb[128:192]])],
                       axis=1)                   # [128, 3]
        in_maps.append({
            "xT": xTb[b],
            "wkq": np.ascontiguousarray(wkq.astype(bf16).reshape(CT, 128, 384)),
            "wv": np.ascontiguousarray(vcols.astype(bf16).reshape(CT, 128, 192)),
            "wp": np.ascontiguousarray(wp.astype(bf16).reshape(2, 128, C)),
            "masks": m_bf,
            "bkq": np.ascontiguousarray(bkq.astype(np.float32)),
        })
    return in_maps


def kernel(x, W_qkv, b_qkv, W_proj, b_proj):
    import os
    from concourse.bass_utils import run_bass_kernel_spmd

    b_qkv = np.asarray(b_qkv, dtype=np.float32)
    b_proj = np.asarray(b_proj, dtype=np.float32)
    W_qkv = np.asarray(W_qkv, dtype=np.float32)
    W_proj = np.asarray(W_proj, dtype=np.float32)
    in_maps = _prep_inputs(x, W_qkv, b_qkv, W_proj, b_proj)
    with_bias = bool(np.any(b_qkv[0:2 * C]))
    if not with_bias:
        for im in in_maps:
            del im["bkq"]
    key = f"nc{with_bias}"
    if key not in _CACHE:
        _CACHE[key] = _build_program(with_bias)
    nc = _CACHE[key]
    res = run_bass_kernel_spmd(nc, in_maps, list(range(NCORES)),
                               trace=os.environ.get("KTRACE", "") == "1")
    _CACHE["last_result"] = res

    # host-side unshard: sum the 4 head-group partials per batch.
    out = np.empty((B, T, C), dtype=np.float32)
    for b in range(B):
        acc = np.zeros((T, C), dtype=np.float32)
        for g in range(4):
            acc += np.asarray(res.results[4 * b + g]["z"]).astype(np.float32)
        # v-bias and proj-bias fold in linearly on the host:
        # out = P(V + bv) Wp + bp = (PV) Wp + bv Wp + bp
        bv = b_qkv[2 * C:3 * C]
        out[b] = acc + (bv @ W_proj + b_proj)[None, :]
    return out


# revision 26
# speedup vs baseline: 1.2838x; 1.0012x over previous
"""Causal self-attention (B=2, T=2048, C=768, H=12) on 8 NeuronCores.

Sharding per the hint: data-parallel on B (cores 0-3 batch 0, 4-7 batch 1)
x tensor-parallel over heads (core d%4 owns heads 3(d%4)..3(d%4)+2). Each
core computes q/k/v for ONLY its 3 heads, its heads' full causal TxT
attention, and a PARTIAL output projection (contraction over its 192 Y
columns). The 4 partials per batch are summed on the host (pure gather +
add) - zero device collectives.

Per-core pipeline (bf16 matmul operands, fp32 PSUM accumulation):
  x^T arrives pre-transposed from host (no on-device transposes at all).
  K^T/Q^T generated in 3 stationary-weight pair-matmuls ([128,2048] each:
  (k0|k1), (k2|q0), (q1|q2)), V in t-major [128, 3, 65] with a folded
  ones-row for the softmax denominator. Attention runs qtile-major
  (256-query tiles) with EXACT causal extents: qtile i processes step
  pairs sp=0..i, each [128, 512] = two 128-key tiles; only the last
  (diagonal) pair needs a mask multiply. exp on ACT (scale=1/8, no
  max-sub), P^T @ [V|1] accumulates [65, 256] per head into a shared
  [65, 768] PSUM tile, reciprocal + ones-matmul broadcast normalizes into
  Y^T, then the projection partial for the qtile's two 128-row t-tiles is
  copied to bf16 and DMAd out. K/Q/V generation is interleaved with the
  qtile loop so ACT/DVE spin up ~6us into the kernel.
"""

import numpy as np
import ml_dtypes

B, T, C, H, D = 2, 2048, 768, 12, 64
NCORES = 8
HPC = 3            # heads per core
QTW = 256          # query tile width
NQT = T // QTW     # 8 query tiles
CT = C // 128      # 6 contraction tiles

_CACHE = {}

# head -> (pair tile index, partition offset) for K^T and Q^T slices.
# pair tiles: 0 = (k0|k1), 1 = (q0|q1), 2 = (k2|q2); tile 3 = q2 copy [64,T]
_KSL = [(0, 0), (0, 64), (2, 0)]
_QSL = [(1, 0), (1, 64), (3, 0)]


def _build_program(with_bias=True):
    import concourse.bass as bass
    import concourse.bacc as bacc
    import concourse.mybir as mybir
    import concourse.tile as tile

    F32 = mybir.dt.float32
    BF16 = mybir.dt.bfloat16
    AF = mybir.ActivationFunctionType

    nc = bacc.Bacc()
    xT_in = nc.declare_dram_parameter("xT", [CT, 128, T], BF16, isOutput=False)
    wkq_in = nc.declare_dram_parameter("wkq", [CT, 128, 384], BF16,
                                       isOutput=False)
    wv_in = nc.declare_dram_parameter("wv", [CT, 128, 192], BF16,
                                      isOutput=False)
    wp_in = nc.declare_dram_parameter("wp", [2, 128, C], BF16, isOutput=False)
    masks_in = nc.declare_dram_parameter("masks", [128, 512], BF16,
                                         isOutput=False)
    if with_bias:
        bkq_in = nc.declare_dram_parameter("bkq", [128, 3], F32,
                                           isOutput=False)
    z_out = nc.declare_dram_parameter("z", [T, C], BF16, isOutput=True)

    scale = 1.0 / float(np.sqrt(D))

    with tile.TileContext(nc) as tc:
        with tc.tile_pool(name="const", bufs=1) as constp, \
             tc.tile_pool(name="data", bufs=1) as datap, \
             tc.tile_pool(name="pt", bufs=3) as ptp, \
             tc.tile_pool(name="small", bufs=3) as smallp, \
             tc.tile_pool(name="zs", bufs=2) as zsp, \
             tc.tile_pool(name="ps", bufs=2, space="PSUM") as psp, \
             tc.tile_pool(name="pg", bufs=2, space="PSUM") as pgp, \
             tc.tile_pool(name="pot", bufs=2, space="PSUM") as potp:

            # ---- constants ------------------------------------------------
            wkq_s = constp.tile([128, CT, 384], BF16, tag="wkq")
            wv_s = constp.tile([128, CT, 192], BF16, tag="wv")
            wp_s = constp.tile([128, 2, C], BF16, tag="wp")
            masks_s = constp.tile([128, 512], BF16, tag="masks")
            ones1 = constp.tile([1, 64], BF16, tag="ones1")
            nc.vector.memset(ones1, 1.0)
            if with_bias:
                bkq_s = constp.tile([128, 3], F32, tag="bkq")

            # ---- persistent data ------------------------------------------
            xT = [datap.tile([128, T], BF16, tag=f"xT{c}", name=f"xT{c}")
                  for c in range(CT)]
            KQ = [datap.tile([128, T], BF16, tag=f"KQ{j}", name=f"KQ{j}")
                  for j in range(3)]
            KQ.append(datap.tile([64, T], BF16, tag="KQ3", name="KQ3"))
            V = [datap.tile([128, HPC, D + 1], BF16, tag=f"V{t}",
                            name=f"V{t}") for t in range(T // 128)]
            YT0 = datap.tile([128, T], BF16, tag="YT0", name="YT0")
            YT1 = datap.tile([64, T], BF16, tag="YT1", name="YT1")

            # ---- input DMAs (priority order) ------------------------------
            # scalar queue: wkq + first xT chunk c-interleaved so the first
            # gen matmul can start ~1us in; sync queue: the xT bulk;
            # SP/gpsimd: remaining weights + masks.
            for c in range(CT):
                nc.scalar.dma_start(
                    out=wkq_s[:, c, :],
                    in_=wkq_in[c, :, :])
                nc.scalar.dma_start(
                    out=xT[c][:, 0:256], in_=xT_in[c, :, 0:256])
            for lo in (256, 1024):
                for c in range(CT):
                    nc.sync.dma_start(
                        out=xT[c][:, lo:lo + 768],
                        in_=xT_in[c, :, lo:lo + 768])
            nc.sync.dma_start(
                out=wv_s,
                in_=bass.AP(tensor=wv_in[:, :, :].tensor,
                            offset=wv_in[:, :, :].offset,
                            ap=[[192, 128], [128 * 192, CT], [1, 192]]))
            nc.sync.dma_start(out=masks_s, in_=masks_in[:, :])
            nc.gpsimd.dma_start(
                out=wp_s,
                in_=bass.AP(tensor=wp_in[:, :, :].tensor,
                            offset=wp_in[:, :, :].offset,
                            ap=[[C, 128], [128 * C, 2], [1, C]]))
            if with_bias:
                nc.gpsimd.dma_start(out=bkq_s, in_=bkq_in[:, :])

            def gen_kq(arg):
                cn, j = arg
                lo = 256 * cn
                acc = pgp.tile([128, 256], F32, tag="acc", name="acc")
                for c in range(CT):
                    nc.tensor.matmul(
                        out=acc,
                        lhsT=wkq_s[:, c, 128 * j:128 * (j + 1)],
                        rhs=xT[c][:, lo:lo + 256],
                        start=(c == 0), stop=(c == CT - 1))
                if with_bias:
                    nc.vector.tensor_scalar_add(
                        KQ[j][:, lo:lo + 256], in0=acc,
                        scalar1=bkq_s[:, j:j + 1])
                else:
                    nc.vector.tensor_copy(out=KQ[j][:, lo:lo + 256],
                                          in_=acc)
                if j == 2:
                    # peel q2 (partitions 64:128 of the (k2|q2) pair) into a
                    # base-0 tile so S(h2) operands share a base partition
                    nc.vector.tensor_copy(out=KQ[3][0:64, lo:lo + 256],
                                          in_=KQ[2][64:128, lo:lo + 256])

            def gen_v(t):
                acc = pgp.tile([128, 256], F32, tag="acc", name="accv")
                for c in range(CT):
                    nc.tensor.matmul(
                        out=acc[:, 0:192],
                        lhsT=xT[c][:, 128 * t:128 * (t + 1)],
                        rhs=wv_s[:, c, :],
                        start=(c == 0), stop=(c == CT - 1))
                nc.vector.tensor_copy(out=V[t][:, :, 0:D], in_=acc[:, 0:192])
                nc.vector.memset(V[t][:, :, D:D + 1], 1.0)

            # ---- main loop: generation interleaved into attention rounds --
            # gen(tc) must land before qtile 2*tc; tc0 runs up front, tc>=1
            # is spread one-group-per-sp-round across qtiles 2tc-2, 2tc-1.
            for j in range(3):
                gen_kq((0, j))
            gen_v(0)
            gen_v(1)
            # qtile i consumes KQ chunk i and V[2i], V[2i+1]; emit chunk i+1
            # and its V pair during qtile i's rounds (one thunk per round).
            gen_sched = {
                i: ([(gen_kq, (i + 1, j)) for j in range(3)] +
                    [(gen_v, 2 * i + 2), (gen_v, 2 * i + 3)])
                for i in range(NQT - 1)
            }

            for i in range(NQT):
                pending = list(gen_sched.get(i, []))
                per_round = max(1, -(-len(pending) // (i + 1)))
                qsl = slice(QTW * i, QTW * (i + 1))
                ot = potp.tile([65, 3 * QTW], F32, tag="ot", name="ot")
                pts = {}
                for sp in range(i + 1):
                    for h in range(HPC):
                        jk, pk = _KSL[h]
                        jq, pq = _QSL[h]
                        sps = psp.tile([128, 512], F32, tag="acc",
                                       name="sps")
                        for half in range(2):
                            klo = 256 * sp + 128 * half
                            nc.tensor.matmul(
                                out=sps[:, 256 * half:256 * (half + 1)],
                                lhsT=KQ[jk][pk:pk + 64, klo:klo + 128],
                                rhs=KQ[jq][pq:pq + 64, qsl],
                                start=True, stop=True)
                        pt = ptp.tile([128, 512], BF16, tag="pt", name="pt")
                        nc.scalar.activation(out=pt, in_=sps, func=AF.Exp,
                                             scale=scale)
                        if sp == i:
                            nc.vector.tensor_mul(pt, pt, masks_s)
                        pts[h] = pt
                    for h in range(HPC):
                        hsl = slice(QTW * h, QTW * (h + 1))
                        for half in range(2):
                            nc.tensor.matmul(
                                out=ot[:, hsl],
                                lhsT=V[2 * sp + half][:, h, :],
                                rhs=pts[h][:, 256 * half:256 * (half + 1)],
                                start=(sp == 0 and half == 0),
                                stop=(sp == i and half == 1),
                                skip_group_check=True)
                    for _ in range(per_round):
                        if pending:
                            fn, arg = pending.pop(0)
                            fn(arg)
                for h in range(HPC):
                    hsl = slice(QTW * h, QTW * (h + 1))
                    rec = smallp.tile([1, QTW], F32, tag="rec", name="rec")
                    nc.vector.reciprocal(out=rec, in_=ot[64:65, hsl])
                    recbf = smallp.tile([1, QTW], BF16, tag="recbf",
                                        name="recbf")
                    nc.vector.tensor_copy(out=recbf, in_=rec)
                    recb = pgp.tile([64, QTW], F32, tag="acc", name="recb")
                    nc.tensor.matmul(out=recb, lhsT=ones1, rhs=recbf,
                                     start=True, stop=True)
                    recb_sb = smallp.tile([64, QTW], F32, tag="recb_sb",
                                          name="recb_sb")
                    nc.vector.tensor_copy(out=recb_sb, in_=recb)
                    ysl = (YT0[0:64, qsl] if h == 0 else
                           YT0[64:128, qsl] if h == 1 else YT1[0:64, qsl])
                    nc.vector.tensor_mul(ysl, ot[0:64, hsl], recb_sb)

                for tt in (2 * i, 2 * i + 1):
                    tsl = slice(128 * tt, 128 * (tt + 1))
                    zt = zsp.tile([128, C], BF16, tag="zt", name="zt")
                    for ph in range(2):
                        csl = slice(384 * ph, 384 * (ph + 1))
                        pacc = pgp.tile([128, 384], F32, tag="acc",
                                        name="pacc")
                        nc.tensor.matmul(out=pacc, lhsT=YT0[:, tsl],
                                         rhs=wp_s[:, 0, csl], start=True,
                                         stop=False)
                        nc.tensor.matmul(out=pacc, lhsT=YT1[:, tsl],
                                         rhs=wp_s[0:64, 1, csl],
                                         start=False, stop=True)
                        nc.vector.tensor_copy(out=zt[:, csl], in_=pacc)
                    nc.gpsimd.dma_start(out=z_out[tsl, :], in_=zt)

    nc.finalize()
    return nc


def _prep_inputs(x, W_qkv, b_qkv, W_proj, b_proj):
    bf16 = ml_dtypes.bfloat16
    x = np.asarray(x, dtype=np.float32)
    W_qkv = np.asarray(W_qkv, dtype=np.float32)
    b_qkv = np.asarray(b_qkv, dtype=np.float32)

    # masks [128, 512]: col 256*d + q valid iff q >= 128*d + p
    p = np.arange(128)[:, None]
    q = np.arange(QTW)[None, :]
    m = np.ones((128, 512), dtype=np.float32)
    m[:, 0:QTW] = q >= p
    m[:, QTW:512] = q >= 128 + p
    m_bf = np.ascontiguousarray(m.astype(bf16))

    xTb = [np.ascontiguousarray(
        x[b].T.astype(bf16).reshape(CT, 128, T)) for b in range(B)]

    in_maps = []
    for d in range(NCORES):
        b, g = d // 4, d % 4
        qcols = W_qkv[:, 192 * g:192 * (g + 1)]
        kcols = W_qkv[:, C + 192 * g:C + 192 * (g + 1)]
        vcols = W_qkv[:, 2 * C + 192 * g:2 * C + 192 * (g + 1)]
        wkq = np.concatenate(
            [kcols[:, 0:128], qcols[:, 0:128], kcols[:, 128:192],
             qcols[:, 128:192]], axis=1)         # [768, 384]
        wp = np.zeros((256, C), dtype=np.float32)
        wp[0:192] = W_proj[192 * g:192 * (g + 1), :]
        qb = b_qkv[192 * g:192 * (g + 1)]
        kb = b_qkv[C + 192 * g:C + 192 * (g + 1)]
        bkq = np.stack([kb[0:128], qb[0:128],
                        np.concatenate([kb[128:192], qb[128:192]])],
                       axis=1)                   # [128, 3]
        in_maps.append({
            "xT": xTb[b],
            "wkq": np.ascontiguousarray(wkq.astype(bf16).reshape(CT, 128, 384)),
            "wv": np.ascontiguousarray(vcols.astype(bf16).reshape(CT, 128, 192)),
            "wp": np.ascontiguousarray(wp.astype(bf16).reshape(2, 128, C)),
            "masks": m_bf,
            "bkq": np.ascontiguousarray(bkq.astype(np.float32)),
        })
    return in_maps


def kernel(x, W_qkv, b_qkv, W_proj, b_proj):
    import os
    from concourse.bass_utils import run_bass_kernel_spmd

    b_qkv = np.asarray(b_qkv, dtype=np.float32)
    b_proj = np.asarray(b_proj, dtype=np.float32)
    W_qkv = np.asarray(W_qkv, dtype=np.float32)
    W_proj = np.asarray(W_proj, dtype=np.float32)
    in_maps = _prep_inputs(x, W_qkv, b_qkv, W_proj, b_proj)
    with_bias = bool(np.any(b_qkv[0:2 * C]))
    if not with_bias:
        for im in in_maps:
            del im["bkq"]
    key = f"nc{with_bias}"
    if key not in _CACHE:
        _CACHE[key] = _build_program(with_bias)
    nc = _CACHE[key]
    res = run_bass_kernel_spmd(nc, in_maps, list(range(NCORES)),
                               trace=os.environ.get("KTRACE", "") == "1")
    _CACHE["last_result"] = res

    # host-side unshard: sum the 4 head-group partials per batch.
    out = np.empty((B, T, C), dtype=np.float32)
    for b in range(B):
        acc = np.zeros((T, C), dtype=np.float32)
        for g in range(4):
            acc += np.asarray(res.results[4 * b + g]["z"]).astype(np.float32)
        # v-bias and proj-bias fold in linearly on the host:
        # out = P(V + bv) Wp + bp = (PV) Wp + bv Wp + bp
        bv = b_qkv[2 * C:3 * C]
        out[b] = acc + (bv @ W_proj + b_proj)[None, :]
    return out
